# revision 1
# baseline (speedup 1.0000x reference)
"""Trainium2 Bass kernel for the per-node compressor + SE-gate + classifier
model. Data-parallel over batch B across 8 NeuronCores (512 rows/core).

Key optimizations over the naive blocking (kernel is PE-bound; bf16 K=128
matmuls cost ~247 ns, K<128 ones ~434 ns on this hardware):
- SE mean matmuls absorbed into the classifier matmuls as extra psum output
  columns 100:118 (wcl carries the x-mean pattern, the per-source "chosen"
  wco group carries the comp-mean pattern and runs early in stage A); the SE
  pre-activation is assembled from 12 [32, BL] slices at the 32-aligned
  partition window 96:128 on DVE/Pool, removing 72 of 708 matmuls.
- Gate matmuls zero-padded to K=128 (active wbe rows at 100:118) to avoid the
  K<128 per-instruction penalty.
- W1 (the largest weight stream, 9.4 MB/rep) stays SBUF-resident across reps;
  pl/po partials and gates are stored bf16 to make room.
- Matmul operands bf16; accumulation and the gated combine stay fp32.

`_build_nc(reps, loop=True)` wraps the body in a For_i hardware loop for
high-SNR timing (see test.py).
"""

import numpy as np

import concourse.bass as bass
import concourse.tile as tile
from concourse import bacc, mybir
from concourse.bass_utils import run_bass_kernel_spmd

B, N, F, FO, C = 4096, 6, 1024, 512, 100
HID = (F + FO) // 2          # 768
RED = N // 2                 # 3
EPS = 1e-5
IDX = np.array([[j for j in range(N) if j != i] for i in range(N)])

NCORES = 8
BL = B // NCORES             # 512
P = 128
KF = F // P                  # 8
MH = HID // P                # 6
KH = HID // P                # 6
MO = FO // P                 # 4
NB = (N - 1) * FO // P       # 20
CE = C + RED * N             # 118
CEP = P                      # psum/classifier tile partitions (32-aligned pad)
AOFF = C - 96                # 4: mean rows sit at [AOFF:AOFF+18] of the [96:128] window

f32 = mybir.dt.float32
bf16 = mybir.dt.bfloat16
MMDT = bf16
AF = mybir.ActivationFunctionType

# chosen consumer for each source j (carries the cc columns): n' = (j+1) % N
CHOSEN = {}
for j in range(N):
    nprime = (j + 1) % N
    kprime = [k for k in range(N - 1) if IDX[nprime][k] == j][0]
    CHOSEN[j] = (nprime, kprime)

LAST_EXEC_TIME_NS = None
_BUILT = {}


def _build_nc(reps=1, loop=False):
    nc = bacc.Bacc("TRN2", target_bir_lowering=False, debug=False,
                   num_devices=NCORES)

    xT_d = nc.dram_tensor("xT", [N, P, KF, BL], MMDT, kind="ExternalInput").ap()
    w1_d = nc.dram_tensor("w1", [N, P, MH, KF, P], MMDT, kind="ExternalInput").ap()
    w2_d = nc.dram_tensor("w2", [N, P, MO, KH, P], MMDT, kind="ExternalInput").ap()
    wcl_d = nc.dram_tensor("wcl", [N, P, KF, CEP], MMDT, kind="ExternalInput").ap()
    wco_d = nc.dram_tensor("wco", [N, P, NB, CEP], MMDT, kind="ExternalInput").ap()
    wbe_d = nc.dram_tensor("wbe", [N, P, N, C], MMDT, kind="ExternalInput").ap()
    t1_d = nc.dram_tensor("t1c", [P, N, MH], f32, kind="ExternalInput").ap()
    t2_d = nc.dram_tensor("t2c", [P, N, MO], f32, kind="ExternalInput").ap()
    bc_d = nc.dram_tensor("bcc", [C, N], f32, kind="ExternalInput").ap()
    out_d = nc.dram_tensor("out", [N, C, BL], f32, kind="ExternalOutput").ap()

    with tile.TileContext(nc) as tc:
        with (
            tc.tile_pool(name="consts", bufs=1) as consts,
            tc.tile_pool(name="xpool", bufs=3) as xpool,
            tc.tile_pool(name="wpool", bufs=3) as wpool,
            tc.tile_pool(name="hpool", bufs=2) as hpool,
            tc.tile_pool(name="cpool", bufs=1) as cpool,
            tc.tile_pool(name="gpool", bufs=2) as gpool,
            tc.tile_pool(name="pp", bufs=2, space="PSUM") as pp,
        ):
            # PE-critical first loads ahead of the constants
            if not loop:
                xsb0 = xpool.tile([P, KF, BL], MMDT, tag="x", name="xsb")
                nc.sync.dma_start(out=xsb0[:, 0:2, :], in_=xT_d[0, :, 0:2])
            # w1 stays SBUF-resident across reps (largest weight stream)
            w1_res = []
            for n in range(N):
                t = consts.tile([P, MH, KF, P], MMDT, tag=f"w1r{n}")
                nc.sync.dma_start(out=t, in_=w1_d[n])
                w1_res.append(t)

            t1_sb = consts.tile([P, N, MH], f32, tag="t1")
            nc.sync.dma_start(out=t1_sb, in_=t1_d)
            t2_sb = consts.tile([P, N, MO], f32, tag="t2")
            nc.sync.dma_start(out=t2_sb, in_=t2_d)
            bc_sb = consts.tile([C, N], f32, tag="bc")
            nc.sync.dma_start(out=bc_sb, in_=bc_d)
            zeros_sb = consts.tile([P, BL], f32, tag="zeros")
            nc.vector.memset(zeros_sb, 0.0)
            warm_sb = consts.tile([1, 1], f32, tag="warm")
            nc.scalar.activation(out=warm_sb, in_=zeros_sb[0:1, 0:1],
                                 func=AF.Sigmoid, scale=1.0)

            import contextlib

            def rep_body(first):
                comp_sb = []
                pl_sb = []
                po_sb = [None] * N

                # ---- Stage A
                for n in range(N):
                    if first and n == 0:
                        xsb = xsb0
                        for kp in range(2, KF, 2):
                            nc.sync.dma_start(out=xsb[:, kp:kp + 2, :],
                                              in_=xT_d[n, :, kp:kp + 2])
                    else:
                        xsb = xpool.tile([P, KF, BL], MMDT, tag="x", name="xsb")
                        for kp in range(0, KF, 2):
                            nc.sync.dma_start(out=xsb[:, kp:kp + 2, :],
                                              in_=xT_d[n, :, kp:kp + 2])

                    # L1: h = relu(W1' @ x + t1)
                    hsb = hpool.tile([P, MH, BL], MMDT, tag="h")
                    for m in range(MH):
                        ph = pp.tile([P, BL], f32, tag="h", bufs=3)
                        for k in range(KF):
                            nc.tensor.matmul(ph, w1_res[n][:, m, k, :],
                                             xsb[:, k, :],
                                             start=(k == 0), stop=(k == KF - 1))
                        nc.scalar.activation(out=hsb[:, m, :], in_=ph,
                                             func=AF.Relu,
                                             bias=t1_sb[:, n, m:m + 1], scale=1.0)

                    # L2: comp = relu(W2' @ h + t2)
                    csb = cpool.tile([P, MO, BL], MMDT, tag=f"comp{n}")
                    for o in range(MO):
                        w2m = wpool.tile([P, KH, P], MMDT, tag="w2")
                        nc.sync.dma_start(out=w2m, in_=w2_d[n, :, o])
                        pc = pp.tile([P, BL], f32, tag="c", bufs=3)
                        for k in range(KH):
                            nc.tensor.matmul(pc, w2m[:, k, :], hsb[:, k, :],
                                             start=(k == 0), stop=(k == KH - 1))
                        nc.vector.scalar_tensor_tensor(
                            csb[:, o, :], pc, t2_sb[:, n, o:o + 1], zeros_sb,
                            mybir.AluOpType.add, mybir.AluOpType.max)
                    comp_sb.append(csb)

                    # local classifier partial (+ x-mean cols 100:118)
                    wcl = wpool.tile([P, KF, CEP], MMDT, tag="wcl")
                    nc.sync.dma_start(out=wcl, in_=wcl_d[n])
                    ppl = pp.tile([CEP, BL], f32, tag="pl", bufs=2)
                    for k in range(KF):
                        nc.tensor.matmul(ppl, wcl[:, k, :], xsb[:, k, :],
                                         start=(k == 0), stop=(k == KF - 1))
                    pl = cpool.tile([CEP, BL], MMDT, tag=f"pl{n}")
                    nc.vector.tensor_copy(pl, ppl)
                    pl_sb.append(pl)

                    # chosen others-group for source j=n runs early (+cc cols)
                    nprime, kprime = CHOSEN[n]
                    wcoc = wpool.tile([P, MO, CEP], MMDT, tag="wcoc", bufs=2)
                    nc.sync.dma_start(
                        out=wcoc,
                        in_=wco_d[nprime, :, kprime * MO:(kprime + 1) * MO])
                    ppo = pp.tile([CEP, BL], f32, tag="pl", bufs=2)
                    for o in range(MO):
                        nc.tensor.matmul(ppo, wcoc[:, o, :], csb[:, o, :],
                                         start=(o == 0), stop=(o == MO - 1))
                    po = cpool.tile([CEP, BL], MMDT, tag=f"po{n}")
                    nc.scalar.activation(out=po, in_=ppo, func=AF.Copy,
                                         scale=1.0)
                    po_sb[n] = po

                # ---- a_pre assembly: everything stays at base partition
                # 96 (32-aligned); rows 96:100 junk classes x zero wbe rows,
                # 100:118 the means, 118:128 zeros
                slices = [pl_sb[n][96:P, :] for n in range(N)] + \
                         [po_sb[n][96:P, :] for n in range(N)]
                half = [slices[:6], slices[6:]]
                eng = [nc.vector, nc.gpsimd]
                chain_out = []
                for h_i, (e, sl) in enumerate(zip(eng, half)):
                    s = gpool.tile([P, BL], f32, tag=f"asum{h_i}", bufs=2)
                    e.tensor_add(s[96:P, :], sl[0], sl[1])
                    for t in sl[2:]:
                        s2 = gpool.tile([P, BL], f32, tag=f"asum{h_i}",
                                        bufs=2)
                        e.tensor_add(s2[96:P, :], s[96:P, :], t)
                        s = s2
                    chain_out.append(s)
                amerge = gpool.tile([P, BL], f32, tag="asum0", bufs=2)
                nc.vector.tensor_add(amerge[96:P, :], chain_out[0][96:P, :],
                                     chain_out[1][96:P, :])
                # zero-padded to K=128 so the gate matmuls avoid the K<128
                # per-instruction penalty; active rows at 100:118
                a_sb = gpool.tile([P, BL], MMDT, tag="a", bufs=2)
                nc.vector.memset(a_sb, 0.0)
                nc.scalar.activation(out=a_sb[96:P, :], in_=amerge[96:P, :],
                                     func=AF.Relu, scale=1.0)

                # ---- Stage C
                for n in range(N):
                    wbe = wpool.tile([P, N, C], MMDT, tag="wbe")
                    nc.sync.dma_start(out=wbe, in_=wbe_d[n])
                    gates = []
                    for m in range(N):
                        pg = pp.tile([C, BL], f32, tag="c", bufs=3)
                        nc.tensor.matmul(pg, wbe[:, m, :], a_sb,
                                         start=True, stop=True)
                        g = gpool.tile([C, BL], MMDT, tag="gate", bufs=7)
                        nc.scalar.activation(out=g, in_=pg, func=AF.Sigmoid,
                                             scale=1.0)
                        gates.append(g)

                    wco = wpool.tile([P, NB, CEP], MMDT, tag="wco", bufs=2)
                    nc.sync.dma_start(out=wco, in_=wco_d[n])

                    terms = []
                    t0 = gpool.tile([C, BL], f32, tag="tmp", bufs=7)
                    nc.gpsimd.tensor_mul(t0, pl_sb[n][0:C, :], gates[0])
                    terms.append(t0)
                    for k in range(N - 1):
                        j = int(IDX[n][k])
                        tmp = gpool.tile([C, BL], f32, tag="tmp", bufs=7)
                        if CHOSEN[j][0] == n:
                            nc.vector.tensor_mul(tmp, po_sb[j][0:C, :],
                                                 gates[k + 1])
                        else:
                            pq = pp.tile([C, BL], f32, tag="h", bufs=3)
                            for o in range(MO):
                                nc.tensor.matmul(pq,
                                                 wco[:, k * MO + o, 0:C],
                                                 comp_sb[j][:, o, :],
                                                 start=(o == 0),
                                                 stop=(o == MO - 1))
                            nc.vector.tensor_mul(tmp, pq, gates[k + 1])
                        terms.append(tmp)
                    eng = [nc.vector, nc.gpsimd]
                    ei = 0
                    while len(terms) > 1:
                        nxt = []
                        for i in range(0, len(terms) - 1, 2):
                            s = gpool.tile([C, BL], f32, tag="tmp", bufs=7)
                            eng[ei % 2].tensor_add(s, terms[i], terms[i + 1])
                            ei += 1
                            nxt.append(s)
                        if len(terms) % 2:
                            nxt.append(terms[-1])
                        terms = nxt

                    osb = gpool.tile([C, BL], f32, tag="osb", bufs=2)
                    nc.gpsimd.tensor_scalar_add(osb, terms[0], bc_sb[:, n:n + 1])
                    nc.sync.dma_start(out=out_d[n], in_=osb)

            if loop:
                # unroll multiple rep bodies per For_i iteration to amortize
                # the all-engine barrier overhead in the timed loop
                unroll = next(u for u in (40, 8, 4, 2, 1) if reps % u == 0)
                with tc.For_i(0, reps // unroll):
                    for _u in range(unroll):
                        rep_body(False)
            else:
                for _rep in range(reps):
                    rep_body(_rep == 0)

    nc.compile()
    return nc


def _host_prep(x, W1, b1, g1, be1, rm1, rv1, W2, b2, g2, be2, rm2, rv2,
               Wa, Wb, Wc, bc):
    import ml_dtypes
    mmnp = ml_dtypes.bfloat16
    f = np.float32
    s1 = (g1 / np.sqrt(rv1 + EPS)).astype(f)
    t1 = ((b1 - rm1) * s1 + be1).astype(f)
    W1f = (W1 * s1[:, :, None]).astype(f)
    s2 = (g2 / np.sqrt(rv2 + EPS)).astype(f)
    t2 = ((b2 - rm2) * s2 + be2).astype(f)
    W2f = (W2 * s2[:, :, None]).astype(f)

    shared = {}
    shared["w1"] = np.ascontiguousarray(
        W1f.reshape(N, MH, P, KF, P).transpose(0, 4, 1, 3, 2))
    shared["w2"] = np.ascontiguousarray(
        W2f.reshape(N, MO, P, KH, P).transpose(0, 4, 1, 3, 2))

    Wa = np.asarray(Wa, dtype=f)
    cxb = np.zeros((N, RED * N), dtype=f)
    ccb = np.zeros((N, RED * N), dtype=f)
    for n in range(N):
        for r in range(RED):
            cxb[n, n * RED + r] = Wa[n, r, 0] / F
        for k in range(N - 1):
            j = int(IDX[n][k])
            for r in range(RED):
                ccb[j, n * RED + r] = Wa[n, r, 1 + k] / FO
    Wc = np.asarray(Wc, dtype=f)
    wcl = np.zeros((N, P, KF, CEP), dtype=f)
    wcl[:, :, :, :C] = Wc[:, :, :F].reshape(N, C, KF, P).transpose(0, 3, 2, 1)
    for n in range(N):
        wcl[n, :, :, C:CE] = cxb[n][None, None, :]
    shared["wcl"] = wcl
    wco = np.zeros((N, P, NB, CEP), dtype=f)
    wco[:, :, :, :C] = Wc[:, :, F:].reshape(N, C, NB, P).transpose(0, 3, 2, 1)
    for j in range(N):
        nprime, kprime = CHOSEN[j]
        for o in range(MO):
            wco[nprime, :, kprime * MO + o, C:CE] = ccb[j][None, :]
    shared["wco"] = wco

    Wb = np.asarray(Wb, dtype=f)
    wbe = np.zeros((N, P, N, C), dtype=f)
    for n in range(N):
        for m in range(N):
            for r in range(RED):
                wbe[n, C + n * RED + r, m, :] = Wb[n, m, r]
    shared["wbe"] = wbe

    shared["t1c"] = np.ascontiguousarray(t1.reshape(N, MH, P).transpose(2, 0, 1))
    shared["t2c"] = np.ascontiguousarray(t2.reshape(N, MO, P).transpose(2, 0, 1))
    shared["bcc"] = np.ascontiguousarray(np.asarray(bc, dtype=f).T)

    for k in ("w1", "w2", "wcl", "wco", "wbe"):
        shared[k] = shared[k].astype(mmnp)

    x = np.asarray(x, dtype=f)
    in_maps = []
    for i in range(NCORES):
        xi = x[i * BL:(i + 1) * BL]
        xt = np.ascontiguousarray(
            xi.transpose(1, 2, 0).reshape(N, KF, P, BL).transpose(0, 2, 1, 3)
        ).astype(mmnp)
        m = dict(shared)
        m["xT"] = xt
        in_maps.append(m)
    return in_maps


def kernel(**inputs):
    global LAST_EXEC_TIME_NS
    if "nc" not in _BUILT:
        _BUILT["nc"] = _build_nc()
    nc = _BUILT["nc"]

    inputs = {k: np.asarray(v) for k, v in inputs.items()}
    in_maps = _host_prep(**inputs)
    res = run_bass_kernel_spmd(nc, in_maps, core_ids=list(range(NCORES)))
    LAST_EXEC_TIME_NS = res.exec_time_ns

    out = np.empty((B, N, C), dtype=np.float32)
    for i in range(NCORES):
        out[i * BL:(i + 1) * BL] = res.results[i]["out"].transpose(2, 0, 1)
    return out



# revision 2
# speedup vs baseline: 1.0152x; 1.0152x over previous
"""Trainium2 Bass kernel: per-node compressor + SE-gate + classifier.
Data-parallel over batch B across 8 NeuronCores (512 rows/core).

Optimizations over the bf16 baseline (213.7 us):
- The compressor layers L1/L2 run as 1-term fp8e4 DoubleRow matmuls
  (2 K-tiles contracted per instruction): per-row power-of-2 stored-weight
  scales keep the fp8 mantissas centered; L1's psum is read out on ACT with
  per-row fp32 scale/bias APs (relu), L2's sig2 scale is folded exactly
  (power-of-2 exponent shifts) into the classifier wco columns so comp's
  readout is a single DVE stt.
- x ships twice: bf16 (classifier accuracy) + fp8 (L1 speed).
- Classifier (with absorbed SE-mean columns at psum rows 100:118), gates
  and the gated combine stay bf16; the combine is a pairwise tree with the
  bias fused into the last pair via stt so only two DVE ops trail the
  final matmul; psum-reading multiplies stay on DVE (Pool cannot access
  PSUM), sbuf-side ops split DVE/Pool.
- Quantization error (validated on hardware): rel err ~1.59e-2 vs the
  2e-2 gate (bf16 baseline was 3.7e-3).
"""

import numpy as np

import concourse.bass as bass
import concourse.tile as tile
from concourse import bacc, mybir
from concourse.bass_utils import run_bass_kernel_spmd

B, N, F, FO, C = 4096, 6, 1024, 512, 100
HID = (F + FO) // 2          # 768
RED = N // 2                 # 3
EPS = 1e-5
IDX = np.array([[j for j in range(N) if j != i] for i in range(N)])

NCORES = 8
BL = B // NCORES             # 512
P = 128
KF = F // P                  # 8
MH = HID // P                # 6
KH = HID // P                # 6
MO = FO // P                 # 4
NB = (N - 1) * FO // P       # 20
CE = C + RED * N             # 118
CEP = P                      # classifier psum partitions (padded)
SX = 32.0                    # x fp8 scale

f32 = mybir.dt.float32
bf16 = mybir.dt.bfloat16
f8 = mybir.dt.float8e4
MMDT = bf16
AF = mybir.ActivationFunctionType
DR = mybir.MatmulPerfMode.DoubleRow

# chosen consumer for each source j (carries the cc columns): n' = (j+1) % N
CHOSEN = {}
for j in range(N):
    nprime = (j + 1) % N
    kprime = [k for k in range(N - 1) if IDX[nprime][k] == j][0]
    CHOSEN[j] = (nprime, kprime)

LAST_EXEC_TIME_NS = None
_BUILT = {}


def _build_nc(reps=1, loop=False):
    nc = bacc.Bacc("TRN2", target_bir_lowering=False, debug=False,
                   num_devices=NCORES)

    xT_d = nc.dram_tensor("xT", [N, P, KF, BL], MMDT, kind="ExternalInput").ap()
    x8_d = nc.dram_tensor("x8", [N, P, KF, BL], f8, kind="ExternalInput").ap()
    w1_d = nc.dram_tensor("w1", [N, P, MH, KF, P], f8, kind="ExternalInput").ap()
    w2_d = nc.dram_tensor("w2", [N, P, MO, KH, P], f8, kind="ExternalInput").ap()
    wcl_d = nc.dram_tensor("wcl", [N, P, KF, CEP], MMDT, kind="ExternalInput").ap()
    wco_d = nc.dram_tensor("wco", [N, P, NB, CEP], MMDT, kind="ExternalInput").ap()
    wbe_d = nc.dram_tensor("wbe", [N, P, N, C], MMDT, kind="ExternalInput").ap()
    c1_d = nc.dram_tensor("c1c", [P, N, MH], f32, kind="ExternalInput").ap()
    t1_d = nc.dram_tensor("t1c", [P, N, MH], f32, kind="ExternalInput").ap()
    t2_d = nc.dram_tensor("t2c", [P, N, MO], f32, kind="ExternalInput").ap()
    bc_d = nc.dram_tensor("bcc", [C, N], f32, kind="ExternalInput").ap()
    out_d = nc.dram_tensor("out", [N, C, BL], f32, kind="ExternalOutput").ap()

    with tile.TileContext(nc) as tc:
        with (
            tc.tile_pool(name="consts", bufs=1) as consts,
            tc.tile_pool(name="xpool", bufs=3) as xpool,
            tc.tile_pool(name="wpool", bufs=3) as wpool,
            tc.tile_pool(name="hpool", bufs=2) as hpool,
            tc.tile_pool(name="cpool", bufs=1) as cpool,
            tc.tile_pool(name="gpool", bufs=2) as gpool,
            tc.tile_pool(name="pp", bufs=2, space="PSUM") as pp,
        ):
            # PE-critical first loads ahead of the constants
            if not loop:
                x8sb0 = xpool.tile([P, KF, BL], f8, tag="x8", name="x8sb")
                nc.sync.dma_start(out=x8sb0[:, 0:2, :], in_=x8_d[0, :, 0:2])
                w1sb0 = wpool.tile([P, MH, KF, P], f8, tag="w1", name="w1sb")
                nc.sync.dma_start(out=w1sb0[:, 0:1], in_=w1_d[0, :, 0:1])

            c1_sb = consts.tile([P, N, MH], f32, tag="c1")
            nc.sync.dma_start(out=c1_sb, in_=c1_d)
            t1_sb = consts.tile([P, N, MH], f32, tag="t1")
            nc.sync.dma_start(out=t1_sb, in_=t1_d)
            t2_sb = consts.tile([P, N, MO], f32, tag="t2")
            nc.sync.dma_start(out=t2_sb, in_=t2_d)
            bc_sb = consts.tile([C, N], f32, tag="bc")
            nc.sync.dma_start(out=bc_sb, in_=bc_d)
            zeros_sb = consts.tile([P, BL], f32, tag="zeros")
            nc.vector.memset(zeros_sb, 0.0)
            warm_sb = consts.tile([1, 1], f32, tag="warm")
            nc.scalar.activation(out=warm_sb, in_=zeros_sb[0:1, 0:1],
                                 func=AF.Sigmoid, scale=1.0)

            def rep_body(first):
                comp_sb = []
                pl_sb = []
                po_sb = [None] * N

                # ---- Stage A
                for n in range(N):
                    if first and n == 0:
                        x8sb = x8sb0
                        nc.sync.dma_start(out=x8sb[:, 2:4, :],
                                          in_=x8_d[n, :, 2:4])
                        nc.sync.dma_start(out=x8sb[:, 4:8, :],
                                          in_=x8_d[n, :, 4:8])
                        w1sb = w1sb0
                        nc.sync.dma_start(out=w1sb[:, 1:2], in_=w1_d[n, :, 1:2])
                        nc.sync.dma_start(out=w1sb[:, 2:6], in_=w1_d[n, :, 2:6])
                    else:
                        x8sb = xpool.tile([P, KF, BL], f8, tag="x8",
                                          name="x8sb")
                        nc.sync.dma_start(out=x8sb, in_=x8_d[n])
                        w1sb = wpool.tile([P, MH, KF, P], f8, tag="w1",
                                          name="w1sb")
                        nc.sync.dma_start(out=w1sb, in_=w1_d[n])
                    # bf16 x for the classifier (needed later in this node)
                    xsb = xpool.tile([P, KF, BL], MMDT, tag="x", name="xsb")
                    nc.sync.dma_start(out=xsb, in_=xT_d[n])

                    # L1: h8 = fp8(relu(c1*psum + t1s)), DoubleRow fp8
                    h8sb = hpool.tile([P, MH, BL], f8, tag="h")
                    for m in range(MH):
                        ph = pp.tile([P, BL], f32, tag="h", bufs=3)
                        for kk in range(KF // 2):
                            nc.tensor.matmul(ph,
                                             w1sb[:, m, 2 * kk:2 * kk + 2, :],
                                             x8sb[:, 2 * kk:2 * kk + 2, :],
                                             start=(kk == 0),
                                             stop=(kk == KF // 2 - 1),
                                             perf_mode=DR)
                        nc.scalar.activation(out=h8sb[:, m, :], in_=ph,
                                             func=AF.Relu,
                                             bias=t1_sb[:, n, m:m + 1],
                                             scale=c1_sb[:, n, m:m + 1])

                    # L2: comp = relu(c2*psum + t2), DoubleRow fp8, bf16 out
                    w2sb = wpool.tile([P, MO, KH, P], f8, tag="w2")
                    nc.sync.dma_start(out=w2sb, in_=w2_d[n])
                    csb = cpool.tile([P, MO, BL], MMDT, tag=f"comp{n}")
                    for o in range(MO):
                        pc = pp.tile([P, BL], f32, tag="c", bufs=3)
                        for kk in range(KH // 2):
                            nc.tensor.matmul(pc,
                                             w2sb[:, o, 2 * kk:2 * kk + 2, :],
                                             h8sb[:, 2 * kk:2 * kk + 2, :],
                                             start=(kk == 0),
                                             stop=(kk == KH // 2 - 1),
                                             perf_mode=DR)
                        nc.vector.scalar_tensor_tensor(
                            csb[:, o, :], pc, t2_sb[:, n, o:o + 1], zeros_sb,
                            mybir.AluOpType.add, mybir.AluOpType.max)
                    comp_sb.append(csb)

                    # local classifier partial (+ x-mean cols 100:118), bf16
                    wcl = wpool.tile([P, KF, CEP], MMDT, tag="wcl")
                    nc.sync.dma_start(out=wcl, in_=wcl_d[n])
                    ppl = pp.tile([CEP, BL], f32, tag="pl", bufs=2)
                    for k in range(KF):
                        nc.tensor.matmul(ppl, wcl[:, k, :], xsb[:, k, :],
                                         start=(k == 0), stop=(k == KF - 1))
                    pl = cpool.tile([CEP, BL], MMDT, tag=f"pl{n}")
                    nc.scalar.activation(out=pl, in_=ppl, func=AF.Copy,
                                         scale=1.0)
                    pl_sb.append(pl)

                    # chosen others-group for source j=n runs early (+cc cols)
                    nprime, kprime = CHOSEN[n]
                    wcoc = wpool.tile([P, MO, CEP], MMDT, tag="wcoc", bufs=2)
                    nc.sync.dma_start(
                        out=wcoc,
                        in_=wco_d[nprime, :, kprime * MO:(kprime + 1) * MO])
                    ppo = pp.tile([CEP, BL], f32, tag="pl", bufs=2)
                    for o in range(MO):
                        nc.tensor.matmul(ppo, wcoc[:, o, :], csb[:, o, :],
                                         start=(o == 0), stop=(o == MO - 1))
                    po = cpool.tile([CEP, BL], MMDT, tag=f"po{n}")
                    nc.scalar.activation(out=po, in_=ppo, func=AF.Copy,
                                         scale=1.0)
                    po_sb[n] = po

                # ---- a_pre assembly at partition window 96:128 (32-aligned);
                # rows 96:100 real classes x zero wbe rows, 100:118 the means,
                # 118:128 zeros
                slices = [pl_sb[n][96:P, :] for n in range(N)] + \
                         [po_sb[n][96:P, :] for n in range(N)]
                half = [slices[:6], slices[6:]]
                eng = [nc.vector, nc.gpsimd]
                chain_out = []
                for h_i, (e, sl) in enumerate(zip(eng, half)):
                    s = gpool.tile([P, BL], f32, tag=f"asum{h_i}", bufs=2)
                    e.tensor_add(s[96:P, :], sl[0], sl[1])
                    for t in sl[2:]:
                        s2 = gpool.tile([P, BL], f32, tag=f"asum{h_i}",
                                        bufs=2)
                        e.tensor_add(s2[96:P, :], s[96:P, :], t)
                        s = s2
                    chain_out.append(s)
                amerge = gpool.tile([P, BL], f32, tag="asum0", bufs=2)
                nc.vector.tensor_add(amerge[96:P, :], chain_out[0][96:P, :],
                                     chain_out[1][96:P, :])
                # zero-padded to K=128 so the gate matmuls avoid the K<128
                # per-instruction penalty; active rows at 100:118
                a_sb = gpool.tile([P, BL], MMDT, tag="a", bufs=2)
                nc.vector.memset(a_sb, 0.0)
                nc.scalar.activation(out=a_sb[96:P, :], in_=amerge[96:P, :],
                                     func=AF.Relu, scale=1.0)

                # ---- Stage C
                for n in range(N):
                    wco = wpool.tile([P, NB, CEP], MMDT, tag="wco", bufs=2)
                    nc.sync.dma_start(out=wco, in_=wco_d[n])
                    wbe = wpool.tile([P, N, C], MMDT, tag="wbe")
                    nc.sync.dma_start(out=wbe, in_=wbe_d[n])
                    gates = []
                    gtags = ["c", "pl"]
                    for m in range(N):
                        pg = pp.tile([C, BL], f32, tag=gtags[m % 2],
                                     bufs=(3 if m % 2 == 0 else 2))
                        nc.tensor.matmul(pg, wbe[:, m, :], a_sb,
                                         start=True, stop=True)
                        g = gpool.tile([C, BL], MMDT, tag="gate", bufs=7)
                        nc.scalar.activation(out=g, in_=pg, func=AF.Sigmoid,
                                             scale=1.0)
                        gates.append(g)

                    # psum-reading multiplies must run on DVE; sbuf-side
                    # ones on Pool. Terms accumulate as they arrive; the final
                    # term fuses with the bias add in one stt on DVE so the
                    # tail after the last matmul is mul + stt + dma.
                    terms = []
                    t0 = gpool.tile([C, BL], f32, tag="tmp", bufs=7)
                    nc.gpsimd.tensor_mul(t0, pl_sb[n][0:C, :], gates[0])
                    terms.append(t0)
                    for k in range(N - 1):
                        j = int(IDX[n][k])
                        tmp = gpool.tile([C, BL], f32, tag="tmp", bufs=7)
                        if CHOSEN[j][0] == n:
                            nc.gpsimd.tensor_mul(tmp, po_sb[j][0:C, :],
                                                 gates[k + 1])
                        else:
                            pq = pp.tile([C, BL], f32, tag="h", bufs=3)
                            for o in range(MO):
                                nc.tensor.matmul(pq,
                                                 wco[:, k * MO + o, 0:C],
                                                 comp_sb[j][:, o, :],
                                                 start=(o == 0),
                                                 stop=(o == MO - 1))
                            nc.vector.tensor_mul(tmp, pq, gates[k + 1])
                        terms.append(tmp)
                    # pairwise tree, bias fused into the last pair: after the
                    # final mul only two DVE ops remain before the output DMA
                    p1 = gpool.tile([C, BL], f32, tag="tmp", bufs=7)
                    nc.gpsimd.tensor_add(p1, terms[0], terms[1])
                    p2 = gpool.tile([C, BL], f32, tag="tmp", bufs=7)
                    nc.vector.tensor_add(p2, terms[2], terms[3])
                    p3 = gpool.tile([C, BL], f32, tag="tmp", bufs=7)
                    nc.vector.scalar_tensor_tensor(
                        p3, terms[4], bc_sb[:, n:n + 1], terms[5],
                        mybir.AluOpType.add, mybir.AluOpType.add)
                    q = gpool.tile([C, BL], f32, tag="tmp", bufs=7)
                    nc.gpsimd.tensor_add(q, p1, p2)
                    osb = gpool.tile([C, BL], f32, tag="osb", bufs=2)
                    nc.vector.tensor_add(osb, q, p3)
                    nc.sync.dma_start(out=out_d[n], in_=osb)

            if loop:
                unroll = next(u for u in (40, 8, 4, 2, 1) if reps % u == 0)
                with tc.For_i(0, reps // unroll):
                    for _u in range(unroll):
                        rep_body(False)
            else:
                for _rep in range(reps):
                    rep_body(_rep == 0)

    nc.compile()
    return nc


def _p2(v):
    return 2.0 ** np.round(np.log2(np.maximum(v, 1e-30)))


def _q8(a, dt):
    return np.clip(a, -448, 448).astype(dt)


def _host_prep(x, W1, b1, g1, be1, rm1, rv1, W2, b2, g2, be2, rm2, rv2,
               Wa, Wb, Wc, bc):
    import ml_dtypes
    mmnp = ml_dtypes.bfloat16
    e4 = ml_dtypes.float8_e4m3fn
    f = np.float32

    s1 = (g1 / np.sqrt(rv1 + EPS)).astype(f)
    t1 = ((b1 - rm1) * s1 + be1).astype(f)
    W1f = (W1 * s1[:, :, None]).astype(f)
    s2 = (g2 / np.sqrt(rv2 + EPS)).astype(f)
    t2 = ((b2 - rm2) * s2 + be2).astype(f)
    W2f = (W2 * s2[:, :, None]).astype(f)

    # fp8 stored scales (per output row) + readout affines
    rms1 = np.sqrt(np.mean(W1f * W1f, axis=2) + 1e-30)
    sig1 = _p2(16.0 / rms1)                                     # [N, HID]
    w1q = _q8(W1f * sig1[:, :, None], e4)
    hstd = np.linalg.norm(W1f, axis=2) + 1e-30
    sh = (24.0 / hstd).astype(f)                                # h8 true scale
    c1 = (sh / (sig1 * SX)).astype(f)
    t1s = (t1 * sh).astype(f)

    W2p = (W2f / sh[:, None, :]).astype(f)
    rms2 = np.sqrt(np.mean(W2p * W2p, axis=2) + 1e-30)
    sig2 = _p2(16.0 / rms2)                                     # [N, FO]
    w2q = _q8(W2p * sig2[:, :, None], e4)
    t2s2 = (t2 * sig2).astype(f)

    shared = {}
    shared["w1"] = np.ascontiguousarray(
        w1q.reshape(N, MH, P, KF, P).transpose(0, 4, 1, 3, 2))
    shared["w2"] = np.ascontiguousarray(
        w2q.reshape(N, MO, P, KH, P).transpose(0, 4, 1, 3, 2))

    Wa = np.asarray(Wa, dtype=f)
    cxb = np.zeros((N, RED * N), dtype=f)
    ccb = np.zeros((N, RED * N), dtype=f)
    for n in range(N):
        for r in range(RED):
            cxb[n, n * RED + r] = Wa[n, r, 0] / F
        for k in range(N - 1):
            j = int(IDX[n][k])
            for r in range(RED):
                ccb[j, n * RED + r] = Wa[n, r, 1 + k] / FO
    Wc = np.asarray(Wc, dtype=f)
    wcl = np.zeros((N, P, KF, CEP), dtype=f)
    wcl[:, :, :, :C] = Wc[:, :, :F].reshape(N, C, KF, P).transpose(0, 3, 2, 1)
    for n in range(N):
        wcl[n, :, :, C:CE] = cxb[n][None, None, :]
    shared["wcl"] = wcl
    wco = np.zeros((N, P, NB, CEP), dtype=f)
    wco[:, :, :, :C] = Wc[:, :, F:].reshape(N, C, NB, P).transpose(0, 3, 2, 1)
    for j in range(N):
        nprime, kprime = CHOSEN[j]
        for o in range(MO):
            wco[nprime, :, kprime * MO + o, C:CE] = ccb[j][None, :]
    # comp is stored scaled by sig2 (exact bf16 exponent shift): divide the
    # consuming columns by the source's per-feature sig2
    for n in range(N):
        for k in range(N - 1):
            j = int(IDX[n][k])
            for o in range(MO):
                wco[n, :, k * MO + o, :] /= sig2[j, o * P:(o + 1) * P][:, None]
    shared["wco"] = wco

    Wb = np.asarray(Wb, dtype=f)
    wbe = np.zeros((N, P, N, C), dtype=f)
    for n in range(N):
        for m in range(N):
            for r in range(RED):
                wbe[n, C + n * RED + r, m, :] = Wb[n, m, r]
    shared["wbe"] = wbe

    shared["c1c"] = np.ascontiguousarray(
        c1.reshape(N, MH, P).transpose(2, 0, 1))
    shared["t1c"] = np.ascontiguousarray(
        t1s.reshape(N, MH, P).transpose(2, 0, 1))
    shared["t2c"] = np.ascontiguousarray(
        t2s2.reshape(N, MO, P).transpose(2, 0, 1))
    shared["bcc"] = np.ascontiguousarray(np.asarray(bc, dtype=f).T)

    for k in ("wcl", "wco", "wbe"):
        shared[k] = shared[k].astype(mmnp)

    x = np.asarray(x, dtype=f)
    x8full = _q8(x * SX, e4).astype(f)
    in_maps = []
    for i in range(NCORES):
        sl = slice(i * BL, (i + 1) * BL)
        m = dict(shared)
        m["xT"] = np.ascontiguousarray(
            x[sl].transpose(1, 2, 0).reshape(N, KF, P, BL).transpose(0, 2, 1, 3)
        ).astype(mmnp)
        m["x8"] = np.ascontiguousarray(
            x8full[sl].transpose(1, 2, 0).reshape(N, KF, P, BL)
            .transpose(0, 2, 1, 3)).astype(e4)
        in_maps.append(m)
    return in_maps


def kernel(**inputs):
    global LAST_EXEC_TIME_NS
    if "nc" not in _BUILT:
        _BUILT["nc"] = _build_nc()
    nc = _BUILT["nc"]

    inputs = {k: np.asarray(v) for k, v in inputs.items()}
    in_maps = _host_prep(**inputs)
    res = run_bass_kernel_spmd(nc, in_maps, core_ids=list(range(NCORES)))
    LAST_EXEC_TIME_NS = res.exec_time_ns

    out = np.empty((B, N, C), dtype=np.float32)
    for i in range(NCORES):
        out[i * BL:(i + 1) * BL] = res.results[i]["out"].transpose(2, 0, 1)
    return out


# revision 5
# speedup vs baseline: 1.2457x; 1.2270x over previous
"""Trainium2 Bass kernel: per-node compressor + SE-gate + classifier.
Data-parallel over batch B across 8 NeuronCores (512 rows/core).

v2: the compressor layers L1/L2 run as 1-term fp8e4 DoubleRow matmuls
(2 K-tiles per instruction, ~1.44x measured PE speedup vs bf16), with
per-row power-of-2 stored-weight scales and fp32 per-row affine readouts
on the ACT engine (scale/bias APs). Everything else (classifier with
absorbed SE-mean columns, gates, combine) stays bf16 as in the baseline.

Quantization error budget (validated vs reference in numpy emulation):
rel err ~1.57e-2 vs the 2e-2 gate.
"""

import numpy as np

import concourse.bass as bass
import concourse.tile as tile
from concourse import bacc, mybir
from concourse.bass_utils import run_bass_kernel_spmd

B, N, F, FO, C = 4096, 6, 1024, 512, 100
HID = (F + FO) // 2          # 768
RED = N // 2                 # 3
EPS = 1e-5
IDX = np.array([[j for j in range(N) if j != i] for i in range(N)])

NCORES = 8
BL = B // NCORES             # 512
P = 128
KF = F // P                  # 8
MH = HID // P                # 6
KH = HID // P                # 6
MO = FO // P                 # 4
NB = (N - 1) * FO // P       # 20
CE = C + RED * N             # 118
CEP = P                      # classifier psum partitions (padded)
SX = 32.0                    # x fp8 scale

f32 = mybir.dt.float32
bf16 = mybir.dt.bfloat16
f8 = mybir.dt.float8e4
MMDT = bf16
AF = mybir.ActivationFunctionType
DR = mybir.MatmulPerfMode.DoubleRow

# chosen consumer for each source j (carries the cc columns): n' = (j+1) % N
CHOSEN = {}
for j in range(N):
    nprime = (j + 1) % N
    kprime = [k for k in range(N - 1) if IDX[nprime][k] == j][0]
    CHOSEN[j] = (nprime, kprime)

LAST_EXEC_TIME_NS = None
_BUILT = {}


def _build_nc(reps=1, loop=False):
    nc = bacc.Bacc("TRN2", target_bir_lowering=False, debug=False,
                   num_devices=NCORES)

    x8_d = nc.dram_tensor("x8", [N, P, KF, BL], f8, kind="ExternalInput").ap()
    xd8_d = nc.dram_tensor("xd8", [N, P, KF, BL], f8, kind="ExternalInput").ap()
    w1_d = nc.dram_tensor("w1", [N, P, MH, KF, P], f8, kind="ExternalInput").ap()
    w2_d = nc.dram_tensor("w2", [N, P, MO, KH, P], f8, kind="ExternalInput").ap()
    wcl_d = nc.dram_tensor("wcl", [N, P, KF, CEP], MMDT, kind="ExternalInput").ap()
    wco_d = nc.dram_tensor("wco", [N, P, NB, CEP], MMDT, kind="ExternalInput").ap()
    wbe_d = nc.dram_tensor("wbe", [N, P, N, C], MMDT, kind="ExternalInput").ap()
    c1_d = nc.dram_tensor("c1c", [P, N, MH], f32, kind="ExternalInput").ap()
    t1_d = nc.dram_tensor("t1c", [P, N, MH], f32, kind="ExternalInput").ap()
    t2_d = nc.dram_tensor("t2c", [P, N, MO], f32, kind="ExternalInput").ap()
    bc_d = nc.dram_tensor("bcc", [C, N], f32, kind="ExternalInput").ap()
    out_d = nc.dram_tensor("out", [N, C, BL], f32, kind="ExternalOutput").ap()

    with tile.TileContext(nc) as tc:
        with (
            tc.tile_pool(name="consts", bufs=1) as consts,
            tc.tile_pool(name="xpool", bufs=3) as xpool,
            tc.tile_pool(name="wpool", bufs=3) as wpool,
            tc.tile_pool(name="hpool", bufs=2) as hpool,
            tc.tile_pool(name="cpool", bufs=1) as cpool,
            tc.tile_pool(name="gpool", bufs=2) as gpool,
            tc.tile_pool(name="pp", bufs=2, space="PSUM") as pp,
        ):
            # PE-critical first loads ahead of the constants
            if not loop:
                x8sb0 = xpool.tile([P, KF, BL], f8, tag="x8", name="x8sb")
                nc.sync.dma_start(out=x8sb0[:, 0:2, :], in_=x8_d[0, :, 0:2])
                w1sb0 = wpool.tile([P, MH, KF, P], f8, tag="w1", name="w1sb")
                nc.sync.dma_start(out=w1sb0[:, 0:1], in_=w1_d[0, :, 0:1])

            c1_sb = consts.tile([P, N, MH], f32, tag="c1")
            nc.sync.dma_start(out=c1_sb, in_=c1_d)
            t1_sb = consts.tile([P, N, MH], f32, tag="t1")
            nc.sync.dma_start(out=t1_sb, in_=t1_d)
            t2_sb = consts.tile([P, N, MO], f32, tag="t2")
            nc.sync.dma_start(out=t2_sb, in_=t2_d)
            bc_sb = consts.tile([C, N], f32, tag="bc")
            nc.sync.dma_start(out=bc_sb, in_=bc_d)
            zeros_sb = consts.tile([P, BL], f32, tag="zeros")
            nc.vector.memset(zeros_sb, 0.0)
            warm_sb = consts.tile([1, 1], f32, tag="warm")
            nc.scalar.activation(out=warm_sb, in_=zeros_sb[0:1, 0:1],
                                 func=AF.Sigmoid, scale=1.0)
            nc.scalar.activation(out=warm_sb, in_=zeros_sb[0:1, 0:1],
                                 func=AF.Relu, scale=1.0)
            nc.scalar.activation(out=warm_sb, in_=zeros_sb[0:1, 0:1],
                                 func=AF.Copy, scale=1.0)
            # PE pstate warmup: dummy bf16 matmuls during the initial DMA-idle
            # window so real work starts at full clock (HAM ramped)
            wz_sb = consts.tile([P, P], MMDT, tag="wz")
            nc.vector.memset(wz_sb, 0.0)
            mz_sb = consts.tile([P, BL], MMDT, tag="mz")
            nc.vector.memset(mz_sb, 0.0)
            if not loop:
                pwarm = pp.tile([P, BL], f32, tag="h", bufs=3)
                for _w in range(14):
                    nc.tensor.matmul(pwarm, wz_sb, mz_sb,
                                     start=(_w == 0), stop=(_w == 13))

            def rep_body(first):
                comp_sb = []
                pl_sb = []
                po_sb = [None] * N

                # ---- Stage A
                for n in range(N):
                    if first and n == 0:
                        x8sb = x8sb0
                        nc.sync.dma_start(out=x8sb[:, 2:4, :],
                                          in_=x8_d[n, :, 2:4])
                        nc.sync.dma_start(out=x8sb[:, 4:8, :],
                                          in_=x8_d[n, :, 4:8])
                        w1sb = w1sb0
                        nc.sync.dma_start(out=w1sb[:, 1:2], in_=w1_d[n, :, 1:2])
                        nc.sync.dma_start(out=w1sb[:, 2:6], in_=w1_d[n, :, 2:6])
                    else:
                        x8sb = xpool.tile([P, KF, BL], f8, tag="x8",
                                          name="x8sb")
                        nc.sync.dma_start(out=x8sb, in_=x8_d[n])
                        w1sb = wpool.tile([P, MH, KF, P], f8, tag="w1",
                                          name="w1sb")
                        nc.sync.dma_start(out=w1sb, in_=w1_d[n])
                    # L1: h8 = fp8(relu(c1*psum + t1s)), DoubleRow fp8
                    h8sb = hpool.tile([P, MH, BL], f8, tag="h")
                    for m in range(MH):
                        ph = pp.tile([P, BL], f32, tag="h", bufs=3)
                        for kk in range(KF // 2):
                            nc.tensor.matmul(ph,
                                             w1sb[:, m, 2 * kk:2 * kk + 2, :],
                                             x8sb[:, 2 * kk:2 * kk + 2, :],
                                             start=(kk == 0),
                                             stop=(kk == KF // 2 - 1),
                                             perf_mode=DR)
                        nc.scalar.activation(out=h8sb[:, m, :], in_=ph,
                                             func=AF.Relu,
                                             bias=t1_sb[:, n, m:m + 1],
                                             scale=c1_sb[:, n, m:m + 1])

                    # L2: comp = relu(c2*psum + t2), DoubleRow fp8, bf16 out
                    w2sb = wpool.tile([P, MO, KH, P], f8, tag="w2")
                    nc.sync.dma_start(out=w2sb, in_=w2_d[n])
                    # fp8 residual of x; the classifier's x is reconstructed
                    # on-chip as x8 + xd8 (= x*SX in bf16; wcl carries the
                    # exact 1/SX exponent shift)
                    xd8sb = xpool.tile([P, KF, BL], f8, tag="xd8", name="xd8sb")
                    nc.sync.dma_start(out=xd8sb, in_=xd8_d[n])
                    xsb = xpool.tile([P, KF, BL], MMDT, tag="x", name="xsb")
                    nc.vector.tensor_add(xsb[:, 0:4, :], x8sb[:, 0:4, :],
                                         xd8sb[:, 0:4, :])
                    nc.gpsimd.tensor_add(xsb[:, 4:8, :], x8sb[:, 4:8, :],
                                         xd8sb[:, 4:8, :])

                    csb = cpool.tile([P, MO, BL], MMDT, tag=f"comp{n}")
                    for o in range(MO):
                        pc = pp.tile([P, BL], f32, tag="c", bufs=3)
                        for kk in range(KH // 2):
                            nc.tensor.matmul(pc,
                                             w2sb[:, o, 2 * kk:2 * kk + 2, :],
                                             h8sb[:, 2 * kk:2 * kk + 2, :],
                                             start=(kk == 0),
                                             stop=(kk == KH // 2 - 1),
                                             perf_mode=DR)
                        nc.vector.scalar_tensor_tensor(
                            csb[:, o, :], pc, t2_sb[:, n, o:o + 1], zeros_sb,
                            mybir.AluOpType.add, mybir.AluOpType.max)
                    comp_sb.append(csb)

                    # local classifier partial (+ x-mean cols 100:118), bf16
                    wcl = wpool.tile([P, KF, CEP], MMDT, tag="wcl")
                    nc.sync.dma_start(out=wcl, in_=wcl_d[n])
                    ppl = pp.tile([CEP, BL], f32, tag="pl", bufs=2)
                    for k in range(KF):
                        nc.tensor.matmul(ppl, wcl[:, k, :], xsb[:, k, :],
                                         start=(k == 0), stop=(k == KF - 1))
                    pl = cpool.tile([CEP, BL], MMDT, tag=f"pl{n}")
                    nc.scalar.activation(out=pl, in_=ppl, func=AF.Copy,
                                         scale=1.0)
                    pl_sb.append(pl)

                    # chosen others-group for source j=n runs early (+cc cols)
                    nprime, kprime = CHOSEN[n]
                    wcoc = wpool.tile([P, MO, CEP], MMDT, tag="wcoc", bufs=2)
                    nc.sync.dma_start(
                        out=wcoc,
                        in_=wco_d[nprime, :, kprime * MO:(kprime + 1) * MO])
                    ppo = pp.tile([CEP, BL], f32, tag="pl", bufs=2)
                    for o in range(MO):
                        nc.tensor.matmul(ppo, wcoc[:, o, :], csb[:, o, :],
                                         start=(o == 0), stop=(o == MO - 1))
                    po = cpool.tile([CEP, BL], MMDT, tag=f"po{n}")
                    nc.scalar.activation(out=po, in_=ppo, func=AF.Copy,
                                         scale=1.0)
                    po_sb[n] = po

                # ---- a_pre assembly at partition window 96:128 (32-aligned);
                # rows 96:100 real classes x zero wbe rows, 100:118 the means,
                # 118:128 zeros
                slices = [pl_sb[n][96:P, :] for n in range(N)] + \
                         [po_sb[n][96:P, :] for n in range(N)]
                half = [slices[:6], slices[6:]]
                eng = [nc.vector, nc.gpsimd]
                chain_out = []
                for h_i, (e, sl) in enumerate(zip(eng, half)):
                    s = gpool.tile([P, BL], f32, tag=f"asum{h_i}", bufs=2)
                    e.tensor_add(s[96:P, :], sl[0], sl[1])
                    for t in sl[2:]:
                        s2 = gpool.tile([P, BL], f32, tag=f"asum{h_i}",
                                        bufs=2)
                        e.tensor_add(s2[96:P, :], s[96:P, :], t)
                        s = s2
                    chain_out.append(s)
                amerge = gpool.tile([P, BL], f32, tag="asum0", bufs=2)
                nc.vector.tensor_add(amerge[96:P, :], chain_out[0][96:P, :],
                                     chain_out[1][96:P, :])
                # zero-padded to K=128 so the gate matmuls avoid the K<128
                # per-instruction penalty; active rows at 100:118
                a_sb = gpool.tile([P, BL], MMDT, tag="a", bufs=2)
                nc.vector.memset(a_sb, 0.0)
                nc.scalar.activation(out=a_sb[96:P, :], in_=amerge[96:P, :],
                                     func=AF.Relu, scale=1.0)

                # ---- Stage C
                for n in range(N):
                    wco = wpool.tile([P, NB, CEP], MMDT, tag="wco", bufs=2)
                    nc.sync.dma_start(out=wco, in_=wco_d[n])
                    wbe = wpool.tile([P, N, C], MMDT, tag="wbe")
                    nc.sync.dma_start(out=wbe, in_=wbe_d[n])
                    gates = []
                    gtags = ["c", "pl"]
                    for m in range(N):
                        pg = pp.tile([C, BL], f32, tag=gtags[m % 2],
                                     bufs=(3 if m % 2 == 0 else 2))
                        nc.tensor.matmul(pg, wbe[:, m, :], a_sb,
                                         start=True, stop=True)
                        g = gpool.tile([C, BL], MMDT, tag="gate", bufs=7)
                        nc.scalar.activation(out=g, in_=pg, func=AF.Sigmoid,
                                             scale=1.0)
                        gates.append(g)

                    # psum-reading multiplies must run on DVE; sbuf-side
                    # ones on Pool. Terms accumulate as they arrive; the final
                    # term fuses with the bias add in one stt on DVE so the
                    # tail after the last matmul is mul + stt + dma.
                    terms = []
                    t0 = gpool.tile([C, BL], f32, tag="tmp", bufs=7)
                    nc.gpsimd.tensor_mul(t0, pl_sb[n][0:C, :], gates[0])
                    terms.append(t0)
                    for k in range(N - 1):
                        j = int(IDX[n][k])
                        tmp = gpool.tile([C, BL], f32, tag="tmp", bufs=7)
                        if CHOSEN[j][0] == n:
                            nc.gpsimd.tensor_mul(tmp, po_sb[j][0:C, :],
                                                 gates[k + 1])
                        else:
                            pq = pp.tile([C, BL], f32, tag="h", bufs=3)
                            for o in range(MO):
                                nc.tensor.matmul(pq,
                                                 wco[:, k * MO + o, 0:C],
                                                 comp_sb[j][:, o, :],
                                                 start=(o == 0),
                                                 stop=(o == MO - 1))
                            nc.vector.tensor_mul(tmp, pq, gates[k + 1])
                        terms.append(tmp)
                    # pairwise tree, bias fused into the last pair: after the
                    # final mul only two DVE ops remain before the output DMA
                    p1 = gpool.tile([C, BL], f32, tag="tmp", bufs=7)
                    nc.gpsimd.tensor_add(p1, terms[0], terms[1])
                    p2 = gpool.tile([C, BL], f32, tag="tmp", bufs=7)
                    nc.vector.tensor_add(p2, terms[2], terms[3])
                    p3 = gpool.tile([C, BL], f32, tag="tmp", bufs=7)
                    nc.vector.scalar_tensor_tensor(
                        p3, terms[4], bc_sb[:, n:n + 1], terms[5],
                        mybir.AluOpType.add, mybir.AluOpType.add)
                    q = gpool.tile([C, BL], f32, tag="tmp", bufs=7)
                    nc.gpsimd.tensor_add(q, p1, p2)
                    osb = gpool.tile([C, BL], f32, tag="osb", bufs=2)
                    nc.vector.tensor_add(osb, q, p3)
                    nc.sync.dma_start(out=out_d[n], in_=osb)

            if loop:
                unroll = next(u for u in (40, 8, 4, 2, 1) if reps % u == 0)
                with tc.For_i(0, reps // unroll):
                    for _u in range(unroll):
                        rep_body(False)
            else:
                for _rep in range(reps):
                    rep_body(_rep == 0)

    nc.compile()
    return nc


def _p2(v):
    return 2.0 ** np.round(np.log2(np.maximum(v, 1e-30)))


def _q8(a, dt):
    return np.clip(a, -448, 448).astype(dt)


def _host_prep(x, W1, b1, g1, be1, rm1, rv1, W2, b2, g2, be2, rm2, rv2,
               Wa, Wb, Wc, bc):
    import ml_dtypes
    mmnp = ml_dtypes.bfloat16
    e4 = ml_dtypes.float8_e4m3fn
    f = np.float32

    s1 = (g1 / np.sqrt(rv1 + EPS)).astype(f)
    t1 = ((b1 - rm1) * s1 + be1).astype(f)
    W1f = (W1 * s1[:, :, None]).astype(f)
    s2 = (g2 / np.sqrt(rv2 + EPS)).astype(f)
    t2 = ((b2 - rm2) * s2 + be2).astype(f)
    W2f = (W2 * s2[:, :, None]).astype(f)

    # fp8 stored scales (per output row) + readout affines
    rms1 = np.sqrt(np.mean(W1f * W1f, axis=2) + 1e-30)
    sig1 = _p2(16.0 / rms1)                                     # [N, HID]
    w1q = _q8(W1f * sig1[:, :, None], e4)
    hstd = np.linalg.norm(W1f, axis=2) + 1e-30
    sh = (24.0 / hstd).astype(f)                                # h8 true scale
    c1 = (sh / (sig1 * SX)).astype(f)
    t1s = (t1 * sh).astype(f)

    W2p = (W2f / sh[:, None, :]).astype(f)
    rms2 = np.sqrt(np.mean(W2p * W2p, axis=2) + 1e-30)
    sig2 = _p2(16.0 / rms2)                                     # [N, FO]
    w2q = _q8(W2p * sig2[:, :, None], e4)
    t2s2 = (t2 * sig2).astype(f)

    shared = {}
    shared["w1"] = np.ascontiguousarray(
        w1q.reshape(N, MH, P, KF, P).transpose(0, 4, 1, 3, 2))
    shared["w2"] = np.ascontiguousarray(
        w2q.reshape(N, MO, P, KH, P).transpose(0, 4, 1, 3, 2))

    Wa = np.asarray(Wa, dtype=f)
    cxb = np.zeros((N, RED * N), dtype=f)
    ccb = np.zeros((N, RED * N), dtype=f)
    for n in range(N):
        for r in range(RED):
            cxb[n, n * RED + r] = Wa[n, r, 0] / F
        for k in range(N - 1):
            j = int(IDX[n][k])
            for r in range(RED):
                ccb[j, n * RED + r] = Wa[n, r, 1 + k] / FO
    Wc = np.asarray(Wc, dtype=f)
    wcl = np.zeros((N, P, KF, CEP), dtype=f)
    wcl[:, :, :, :C] = Wc[:, :, :F].reshape(N, C, KF, P).transpose(0, 3, 2, 1)
    for n in range(N):
        wcl[n, :, :, C:CE] = cxb[n][None, None, :]
    wcl /= SX          # x arrives scaled by SX (= x8 + xd8); exact 2^-5 shift
    shared["wcl"] = wcl
    wco = np.zeros((N, P, NB, CEP), dtype=f)
    wco[:, :, :, :C] = Wc[:, :, F:].reshape(N, C, NB, P).transpose(0, 3, 2, 1)
    for j in range(N):
        nprime, kprime = CHOSEN[j]
        for o in range(MO):
            wco[nprime, :, kprime * MO + o, C:CE] = ccb[j][None, :]
    # comp is stored scaled by sig2 (exact bf16 exponent shift): divide the
    # consuming columns by the source's per-feature sig2
    for n in range(N):
        for k in range(N - 1):
            j = int(IDX[n][k])
            for o in range(MO):
                wco[n, :, k * MO + o, :] /= sig2[j, o * P:(o + 1) * P][:, None]
    shared["wco"] = wco

    Wb = np.asarray(Wb, dtype=f)
    wbe = np.zeros((N, P, N, C), dtype=f)
    for n in range(N):
        for m in range(N):
            for r in range(RED):
                wbe[n, C + n * RED + r, m, :] = Wb[n, m, r]
    shared["wbe"] = wbe

    shared["c1c"] = np.ascontiguousarray(
        c1.reshape(N, MH, P).transpose(2, 0, 1))
    shared["t1c"] = np.ascontiguousarray(
        t1s.reshape(N, MH, P).transpose(2, 0, 1))
    shared["t2c"] = np.ascontiguousarray(
        t2s2.reshape(N, MO, P).transpose(2, 0, 1))
    shared["bcc"] = np.ascontiguousarray(np.asarray(bc, dtype=f).T)

    for k in ("wcl", "wco", "wbe"):
        shared[k] = shared[k].astype(mmnp)

    x = np.asarray(x, dtype=f)
    x8full = _q8(x * SX, e4).astype(f)
    xd8full = _q8(x * SX - x8full, e4).astype(f)
    in_maps = []
    for i in range(NCORES):
        sl = slice(i * BL, (i + 1) * BL)
        m = dict(shared)
        m["x8"] = np.ascontiguousarray(
            x8full[sl].transpose(1, 2, 0).reshape(N, KF, P, BL)
            .transpose(0, 2, 1, 3)).astype(e4)
        m["xd8"] = np.ascontiguousarray(
            xd8full[sl].transpose(1, 2, 0).reshape(N, KF, P, BL)
            .transpose(0, 2, 1, 3)).astype(e4)
        in_maps.append(m)
    return in_maps


def kernel(**inputs):
    global LAST_EXEC_TIME_NS
    if "nc" not in _BUILT:
        _BUILT["nc"] = _build_nc()
    nc = _BUILT["nc"]

    inputs = {k: np.asarray(v) for k, v in inputs.items()}
    in_maps = _host_prep(**inputs)
    res = run_bass_kernel_spmd(nc, in_maps, core_ids=list(range(NCORES)))
    LAST_EXEC_TIME_NS = res.exec_time_ns

    out = np.empty((B, N, C), dtype=np.float32)
    for i in range(NCORES):
        out[i * BL:(i + 1) * BL] = res.results[i]["out"].transpose(2, 0, 1)
    return out
